# revision 2
# baseline (speedup 1.0000x reference)
"""Discrete-HMM forward-backward (log-space posteriors) on 8 TRN2 NeuronCores.

v2: 16-bit datapath + streamed DMAs + dual-engine multiply.

Problem: B=64, T=4096, K=32.
  alpha_t = logsumexp_i(alpha_{t-1,i} + lA[i,j]) + em_t   (forward)
  beta_t  = logsumexp_j(beta_{t+1,j} + lA[i,j] + em_{t+1,j})  (backward)
  out = log_softmax(alpha + beta, axis=-1)

Strategy (per core, batch-sharded 8 ways -> 8 batch rows/core):
  * Work in exp space: a_t = (a_{t-1} @ A) * e_t ; u_t = e_t * (A @ u_{t+1})
    with e_t = exp(em'), em' = em - max_j(em) + c0 (host preconditioning;
    per-(b,t) shifts cancel in the final K-normalization).
  * Split T into C=256 chunks of L=16. Both directions and all chunks run
    in ONE [128, 1024] step per scan iteration: partitions pack
    (dir, batch-parity, K=32) against block-diagonal fp16 weights
    (A, A, A^T, A^T); columns pack (batch-pair, chunk). W=8 warmup steps
    per chunk exploit HMM mixing to forget the unknown chunk-boundary state.
  * Sequence boundaries are handled inverse-free: chunk 0 fwd (chunk C-1
    bwd) warm up through ones-emissions, then a host-solved "landing" slice
    divides by the (host-simulated, quantization-exact) warmup state so the
    state entering the kept region equals a_0 = pi*e_0 (resp. u_{T-1} =
    e_{T-1}) exactly; kept-step time indices for those chunks shift by one
    and the host writes the exact t=0 / t=T-1 rows itself.
  * dtypes: emissions fp16 (escan), weights fp16, state/history bf16
    (bf16 has f32 range -- the state must not underflow), PSUM f32.
    Matmuls run at 1 col/cycle; DMA volume is halved twice (in fp16,
    out bf16).
  * The scan runs as two independent column chains (cols 0:512 / 512:1024);
    each step is one matmul (PE) + one elementwise multiply. Chain A
    multiplies on the Vector engine, chain B via nc.any (lands on ACT when
    DVE is busy), so the two chains' multiplies overlap and the PE
    ping-pongs between chains. escan streams in per-step slices; kept
    history slices stream out as soon as both chains wrote them.
    gamma = alpha*beta, the -em' shift, and the K-normalization run on the
    host in float64.

kernel(**inputs) takes FULL inputs, returns FULL [64, 4096, 32] float32.
"""

from contextlib import ExitStack

import ml_dtypes
import numpy as np

import concourse.bass as bass
import concourse.bacc as bacc
import concourse.tile as tile
from concourse import mybir
from concourse.bass_utils import run_bass_kernel_spmd

F32 = mybir.dt.float32
F16 = mybir.dt.float16
BF16 = mybir.dt.bfloat16
NPF16 = np.float16
NPBF16 = ml_dtypes.bfloat16

B, T, K = 64, 4096, 32
NCORES = 8
BLOC = B // NCORES            # 8 batches per core
C = 256                       # chunks per core
L = T // C                    # 16 steps per chunk
W = 8                         # warmup steps
S = L + W                     # 24 sequential scan steps
NH = 1024                     # cols: n = bh*C + c, bh in [0,4), c in [0,C)
HC = NH // 2                  # per-chain columns

_BUILT = {}


# ----------------------------------------------------------------------------
# host-side preparation
# ----------------------------------------------------------------------------

def _bf(x):
    return np.asarray(x, np.float32).astype(NPBF16).astype(np.float32)


def _host_prep(emission_logp, log_pi, log_A):
    f32 = np.float32
    em = np.asarray(emission_logp, dtype=f32)
    log_pi = np.asarray(log_pi, dtype=np.float64)
    log_A = np.asarray(log_A, dtype=np.float64)

    lp = log_pi - np.log(np.sum(np.exp(log_pi)))
    lA = log_A - np.log(np.sum(np.exp(log_A), axis=1, keepdims=True))
    A = np.exp(lA).astype(f32)          # [K,K], rows sum to 1
    pi = np.exp(lp)                     # f64

    # precondition emissions: e_t <= e^{c0}, ~zero mean log-drift per step
    m = em.max(axis=-1, keepdims=True)
    c0 = -np.mean(np.log(np.sum(np.exp(em - m), axis=-1) / K))
    emp = (em - m + c0).astype(f32)     # [B,T,K]
    et = np.exp(emp).astype(f32)

    # quantized weights, exactly as the device will see them
    Aq = A.astype(NPF16).astype(f32)

    # landing slices: simulate the device warmup (bf16 state, fp16 weights,
    # f32 accumulate) through W-1 ones-steps, then solve the landing
    # multiplier so the state after warmup equals the exact boundary value.
    z = np.full(K, 1.0 / K, f32)
    for _ in range(W - 1):
        z = _bf(z @ Aq)
    zA = (z @ Aq).astype(np.float64)              # fwd landing matmul output
    w = np.full(K, 1.0 / K, f32)
    for _ in range(W - 1):
        w = _bf(Aq @ w)
    wA = (Aq @ w).astype(np.float64)              # bwd landing matmul output

    a0 = (pi[None, :] * np.exp(emp[:, 0, :].astype(np.float64)))   # [B,K]
    uT = np.exp(emp[:, T - 1, :].astype(np.float64))               # [B,K]
    land_f = (a0 / zA[None, :]).astype(f32)
    land_b = (uT / wA[None, :]).astype(f32)

    # padded per-(b,j) time series with boundary-shifted gather indices
    cs = np.arange(C)
    ss = np.arange(S)
    tf = (cs[None, :] * L) + ss[:, None] - W      # [S, C]
    tf[:, 0] += 1                                 # chunk-0 shift
    tb = (cs[None, :] * L) + (L + W - 1 - ss[:, None])
    tb[:, C - 1] -= 1                             # chunk-(C-1) shift
    tidx = np.stack([np.clip(tf, 0, T - 1), np.clip(tb, 0, T - 1)],
                    axis=1)                       # [S, 2, C]
    gat = et[:, tidx.reshape(-1), :].reshape(B, S, 2, C, K).copy()

    # chunk-0 fwd / chunk-(C-1) bwd warmup columns: ones then landing
    gat[:, :W - 1, 0, 0, :] = 1.0
    gat[:, W - 1, 0, 0, :] = land_f
    gat[:, :W - 1, 1, C - 1, :] = 1.0
    gat[:, W - 1, 1, C - 1, :] = land_b

    # scan-order buffer escan[core, p, s, n]:
    #   p = dir*64 + bpar*32 + j ; n = bh*C + c ; b = core*8 + bh*2 + bpar
    gat = gat.reshape(NCORES, 4, 2, S, 2, C, K)   # [core,bh,bpar,s,dir,c,j]
    gat = gat.transpose(0, 4, 2, 6, 3, 1, 5)      # [core,dir,bpar,j,s,bh,c]
    escan = np.ascontiguousarray(
        gat.reshape(NCORES, 128, S, NH), dtype=NPF16)

    w4 = np.zeros((128, 128), NPF16)
    for q, M in enumerate([Aq, Aq, Aq.T, Aq.T]):
        w4[32 * q:32 * q + 32, 32 * q:32 * q + 32] = M.astype(NPF16)
    return escan, emp, a0, uT, {"w4": w4}


def _host_post(outs, emp, a0, uT):
    """outs: 8 arrays [128, L*NH] bf16 (history dump) -> log-gamma [B,T,K]."""
    arr = np.stack([o.view(NPBF16) if o.dtype == np.uint16 else o
                    for o in outs], axis=0)
    arr = np.asarray(arr, dtype=np.float64).reshape(
        NCORES, 2, 2, K, L, 4, C)
    # dims: [core, dir, bpar, j, l, bh, c]
    al = arr[:, 0]                                # alpha(t = c*L + l)
    u = arr[:, 1, :, :, ::-1]                     # u, l-reversed -> t = c*L+l
    al = al.transpose(0, 4, 1, 5, 3, 2).reshape(B, T, K)
    u = u.transpose(0, 4, 1, 5, 3, 2).reshape(B, T, K)
    # boundary fixes: chunk-0 fwd history holds a_1..a_L at l=0..L-1
    al[:, 1:L] = al[:, 0:L - 1]
    al[:, 0] = a0
    # chunk-(C-1) bwd history holds u_{T-1-L}.. shifted by one
    u[:, T - L:T - 1] = u[:, T - L + 1:T]
    u[:, T - 1] = uT
    lg = np.log(al) + np.log(u) - emp             # log(alpha*beta) + const
    mx = lg.max(axis=-1, keepdims=True)
    lse = np.log(np.sum(np.exp(lg - mx), axis=-1, keepdims=True)) + mx
    return np.ascontiguousarray((lg - lse).astype(np.float32))


# ----------------------------------------------------------------------------
# bass program (SPMD, one NeuronCore)
# ----------------------------------------------------------------------------

def _build(loop_n=1):
    key = loop_n
    if key in _BUILT:
        return _BUILT[key]

    nc = bacc.Bacc(None, target_bir_lowering=False)

    escan_d = nc.declare_dram_parameter("escan", [128, S, NH], F16,
                                        isOutput=False)
    w4_d = nc.declare_dram_parameter("w4", [128, 128], F16, isOutput=False)
    out_d = nc.declare_dram_parameter("out", [128, L * NH], BF16,
                                      isOutput=True)

    # two independent column chains, each one matmul (PE) + one multiply
    # (DVE) per step; the chains ping-pong so PE and DVE overlap. Routing
    # part of the multiplies through an ACT PSUM->SBUF copy lowers DVE busy
    # but lengthens the state-recursion critical path and does not beat
    # this configuration (the kernel sits at the DMA wall either way).
    CHAINS = [(0, 512, False), (512, 512, False)]

    with tile.TileContext(nc) as tc:
        with ExitStack() as ctx:
            singles = ctx.enter_context(tc.tile_pool(name="singles", bufs=1))
            spool = ctx.enter_context(tc.tile_pool(name="state", bufs=2))
            ppool = ctx.enter_context(
                tc.tile_pool(name="psum", bufs=2, space="PSUM"))

            w4 = singles.tile([128, 128], F16)
            nc.sync.dma_start(out=w4[:], in_=w4_d[:, :])

            esc = [singles.tile([128, NH], F16, tag=f"esc{s}",
                                name=f"esc{s}") for s in range(S)]
            hist = [singles.tile([128, NH], BF16, tag=f"h{l}",
                                 name=f"h{l}") for l in range(L)]

            def body():
                for s in range(S):
                    nc.sync.dma_start(out=esc[s][:], in_=escan_d[:, s, :])

                zc = []
                for h, (c0, cw, _via) in enumerate(CHAINS):
                    z0 = spool.tile([128, cw], BF16, tag=f"zc{h}",
                                    name=f"zc{h}")
                    nc.gpsimd.memset(z0[:], 1.0 / K)
                    zc.append(z0[:])

                for s in range(S):
                    l = s - W
                    for h, (c0, cw, via_act) in enumerate(CHAINS):
                        ps = ppool.tile([128, cw], F32, tag=f"ps{h}",
                                        name=f"ps{h}")
                        nc.tensor.matmul(ps[:], w4[:], zc[h],
                                         start=True, stop=True)
                        if s >= W:
                            dst = hist[l][:, c0:c0 + cw]
                        else:
                            zt = spool.tile([128, cw], BF16, tag=f"zc{h}",
                                            name=f"zt{h}")
                            dst = zt[:]
                        if via_act:
                            cp = spool.tile([128, cw], BF16, tag=f"cp{h}",
                                            name=f"cp{h}")
                            nc.scalar.copy(cp[:], ps[:])
                            nc.vector.tensor_mul(dst, cp[:],
                                                 esc[s][:, c0:c0 + cw])
                        else:
                            nc.vector.tensor_mul(dst, ps[:],
                                                 esc[s][:, c0:c0 + cw])
                        zc[h] = dst
                    if s >= W:
                        nc.sync.dma_start(
                            out=out_d[:, l * NH:(l + 1) * NH],
                            in_=hist[l][:])

            for _rep in range(loop_n):
                body()

    nc.finalize()
    _BUILT[key] = (nc,)
    return _BUILT[key]


# ----------------------------------------------------------------------------
# entry points
# ----------------------------------------------------------------------------

def _run(emission_logp, log_pi, log_A, loop_n=1):
    escan, emp, a0, uT, consts = _host_prep(emission_logp, log_pi, log_A)
    (nc,) = _build(loop_n)
    in_maps = []
    for i in range(NCORES):
        m = {"escan": np.ascontiguousarray(escan[i])}
        m.update(consts)
        in_maps.append(m)
    res = run_bass_kernel_spmd(nc, in_maps, list(range(NCORES)))
    out = _host_post([res.results[i]["out"] for i in range(NCORES)],
                     emp, a0, uT)
    return out, res


def kernel(emission_logp, log_pi, log_A):
    out, _ = _run(emission_logp, log_pi, log_A)
    return out
